# revision 1
# baseline (speedup 1.0000x reference)
"""Differential attention kernel for Trainium2 (8 NeuronCores).

Sharding: 2 batches x 4 V-dim shards (8192 -> 2048 per core). Each core
computes its batch's full attention maps (cheap) and its 2048-wide slice of
V / out_proj; host sums the 4 partial out-projections per batch.

All matmuls run as float32r (fp32 storage, full-rate PE mode). Softmax is
computed without max-subtraction (scores are bounded: |s*scale| < ~15, safe
in fp32). The 1/sum(e1) normalizer is folded into the final out-proj PSUM
eviction (everything after the differential combine is linear, and it is a
per-q diagonal), so the combine is just p = e1 - e2 * (lam*s1/s2).
bqkv is applied on-device via the ScalarE bias port; bv/bo are folded in
exactly on the host using sum_k(diff_attn[q,:]) == 1 - lambda.
"""

import math

import numpy as np

import concourse.bass as bass
from concourse import bacc
import concourse.mybir as mybir
import concourse.tile as tile
from concourse import bass_utils
from concourse.bass import ts, ds
from concourse.masks import make_identity

# Problem shapes (hardcoded per harness contract).
B = 2
S = 2048
D = 512
DQK = 256            # width of each of Q1/Q2/K1/K2
VDIM = 8192
DM = 512             # output dim
NV = 4               # v-shards
VS = VDIM // NV      # 2048 per core
P = 128
QC = 512             # q-chunk
SCALE = 1.0 / math.sqrt(64.0)
LAMBDA_INIT = 0.8
LAYER_INDEX = 0

F32 = mybir.dt.float32
F32R = mybir.dt.float32r
EXP = mybir.ActivationFunctionType.Exp
IDENT = mybir.ActivationFunctionType.Identity
AXX = mybir.AxisListType.X

KD = D // P          # 4 contraction chunks of the input dim
MQ = (2 * D) // P    # 8 m-chunks of qkv output dim
SN = S // 512        # 4 free chunks of S
NKC = S // P         # 16 k-chunks of 128
NVC = VS // P        # 16 v-chunks of 128
NQC = S // QC        # 4 q-chunks
QB = QC // P         # 4 q-blocks per chunk


def kernel_body(tc, xT, wqkv, wv, wo, lam, bq, out, phases="full"):
    nc = tc.nc
    # tolerate f32-typed dram tensors (e.g. run_kernel's sim harness)
    if xT.dtype != F32R:
        xT = xT.bitcast(F32R)
    if wqkv.dtype != F32R:
        wqkv = wqkv.bitcast(F32R)
    if wv.dtype != F32R:
        wv = wv.bitcast(F32R)
    if wo.dtype != F32R:
        wo = wo.bitcast(F32R)
    with (
        tc.tile_pool(name="persist", bufs=1) as persist,
        tc.tile_pool(name="dram", bufs=1, space="DRAM") as dram,
    ):
        _kernel_inner(tc, nc, persist, dram, xT, wqkv, wv, wo, lam, bq, out, phases)


def _kernel_inner(tc, nc, persist, dram, xT, wqkv, wv, wo, lam, bq, out, phases="full"):
    qkvT = persist.tile([P, MQ, S], F32R)     # [d|m-chunks, s]; m: Q1,Q1,Q2,Q2,K1,K1,K2,K2
    lam_sb = persist.tile([P, 1], F32)
    bq_sb = persist.tile([P, MQ], F32)
    ident_f32 = persist.tile([P, P], F32)
    ident = persist.tile([P, P], F32R)

    nc.sync.dma_start(lam_sb, lam)
    nc.sync.dma_start(bq_sb, bq)
    make_identity(nc, ident_f32)
    nc.vector.tensor_copy(ident, ident_f32)

    vd = dram.tile([NVC, P, NKC, P], F32R)  # V panels in [vc][k_in, k_out, v] layout

    # ---------------- setup: qkvT and V ----------------
    with (
        tc.tile_pool(name="setup", bufs=1) as setup,
        tc.tile_pool(name="sbounce", bufs=1) as sbounce,
        tc.tile_pool(name="spsum", bufs=6, space="PSUM") as spsum,
    ):
        xTs = setup.tile([P, KD, S], F32R)
        wq_sb = setup.tile([P, KD, 2 * D], F32R)
        wv_sb = setup.tile([P, KD, VS], F32R)
        # split input loads finely, in first-consumption order: the first
        # psum group (sn=0) needs wq m-half 0 of every dc and xTs[:, dc, sn0]
        for dc in range(KD):
            nc.sync.dma_start(wq_sb[:, dc, :D], wqkv[ds(dc * P, P), :D])
            nc.sync.dma_start(xTs[:, dc, ts(0, 512)], xT[ds(dc * P, P), ts(0, 512)])
        for dc in range(KD):
            nc.sync.dma_start(wq_sb[:, dc, D:], wqkv[ds(dc * P, P), D:])
        for sn in range(1, SN):
            for dc in range(KD):
                nc.sync.dma_start(xTs[:, dc, ts(sn, 512)],
                                  xT[ds(dc * P, P), ts(sn, 512)])
        for dc in range(KD):
            nc.sync.dma_start(wv_sb[:, dc], wv[ds(dc * P, P), :])

        # qkvT[m*128+p, s] = sum_d Wqkv[d, m*128+p] * xT[d, s] + bqkv
        for sn in range(SN):
            for m in range(MQ):
                pt = spsum.tile([P, 512], F32, tag="ps")
                for dc in range(KD):
                    nc.tensor.matmul(
                        pt, wq_sb[:, dc, ts(m, P)], xTs[:, dc, ts(sn, 512)],
                        start=(dc == 0), stop=(dc == KD - 1))
                nc.scalar.activation(qkvT[:, m, ts(sn, 512)], pt, IDENT,
                                     bias=bq_sb[:, m : m + 1])
        # V[s, v] = sum_d x[s, d] Wv[d, v]; staged 4 k-blocks at a time so
        # the panel-layout DRAM writes use 2KB per-partition lines
        NVN = VS // 512
        for g in range(S // P // 4):
            stages = [sbounce.tile([P, 4, 512], F32R, tag=f"st{vn}",
                                   name=f"st_{g}_{vn}") for vn in range(NVN)]
            for smj in range(4):
                sm = g * 4 + smj
                for vn in range(NVN):
                    pt = spsum.tile([P, 512], F32, tag="ps")
                    for dc in range(KD):
                        nc.tensor.matmul(
                            pt, xTs[:, dc, ts(sm, P)], wv_sb[:, dc, ts(vn, 512)],
                            start=(dc == 0), stop=(dc == KD - 1))
                    nc.vector.tensor_copy(stages[vn][:, smj], pt)
            for vn in range(NVN):
                for j in range(4):
                    nc.sync.dma_start(
                        vd[vn * 4 + j, :, ds(g * 4, 4), :],
                        stages[vn][:, :, ts(j, P)])

    if phases == "setup":
        # debug consumer to defeat DCE
        dbg = persist.tile([P, 512], F32, name="dbg")
        nc.vector.tensor_copy(dbg, qkvT[:, 0, :512])
        nc.sync.dma_start(out[ds(0, P), :], dbg)
        nc.sync.dma_start(out[ds(P, P), :].bitcast(F32R), vd[0, :, 0, :].broadcast_to((P, 512)) if False else vd[0].rearrange("p kc v -> p (kc v)")[:, :512])
        return

    # ---------------- main: attention per q-chunk ----------------
    with (
        tc.tile_pool(name="e1p", bufs=2) as e1p,
        tc.tile_pool(name="e2p", bufs=2) as e2p,
        tc.tile_pool(name="tmpp", bufs=4) as tmpp,
        tc.tile_pool(name="smallp", bufs=3) as smallp,
        tc.tile_pool(name="r1p", bufs=2 * QB) as r1p,
        tc.tile_pool(name="ptp", bufs=1) as ptp,
        tc.tile_pool(name="vpp", bufs=3) as vpp,
        tc.tile_pool(name="otp", bufs=3) as otp,
        tc.tile_pool(name="ofp", bufs=2) as ofp,
        tc.tile_pool(name="wop", bufs=1) as wop,
        tc.tile_pool(name="wps", bufs=4, space="PSUM") as wps,
        tc.tile_pool(name="fps", bufs=4, space="PSUM") as fps,
    ):
        woT = wop.tile([P, NVC, DM], F32R)
        nc.sync.dma_start(woT, wo.rearrange("(vc p) m -> p vc m", p=P))
        for qi in range(NQC):
            ptile = ptp.tile([P, NKC, QC], F32R, tag="pt")
            r1s = []
            pend = []   # deferred combine+transpose work, one entry per qb

            def emit_scores(qb):
                qg = qi * QB + qb
                ets = []
                sums = []
                for mi in range(2):
                    qoff, koff = 2 * mi, 4 + 2 * mi
                    pool = e1p if mi == 0 else e2p
                    et = pool.tile([P, S], F32R, tag=f"e{mi}", name=f"e{mi}_{qi}_{qb}")
                    st = smallp.tile([P, SN], F32, tag=f"sum{mi}",
                                     name=f"sum{mi}_{qi}_{qb}")
                    for kn in range(SN):
                        pt = wps.tile([P, 512], F32, tag="ps", name=f"ps_{qi}_{qb}_{mi}_{kn}")
                        for dc in range(2):
                            nc.tensor.matmul(
                                pt,
                                qkvT[:, qoff + dc, ts(qg, P)],
                                qkvT[:, koff + dc, ts(kn, 512)],
                                start=(dc == 0), stop=(dc == 1))
                        nc.scalar.activation(
                            et[:, ts(kn, 512)], pt, EXP, scale=SCALE,
                            accum_out=st[:, kn : kn + 1])
                    ets.append(et)
                    sums.append(st)
                # normalizers: r1 = 1/s1 folded into final out-proj evict;
                # combine uses r2q = lam * s1 / s2.
                s1 = smallp.tile([P, 1], F32, tag="s1", name=f"s1_{qi}_{qb}")
                nc.vector.reduce_sum(s1, sums[0], axis=AXX)
                r1 = r1p.tile([P, 1], F32, tag="r1", name=f"r1_{qi}_{qb}")
                nc.vector.reciprocal(r1, s1)
                r1s.append(r1)
                s2 = smallp.tile([P, 1], F32, tag="s2", name=f"s2_{qi}_{qb}")
                nc.vector.reduce_sum(s2, sums[1], axis=AXX)
                r2 = smallp.tile([P, 1], F32, tag="r2", name=f"r2_{qi}_{qb}")
                nc.vector.reciprocal(r2, s2)
                u = smallp.tile([P, 1], F32, tag="u", name=f"u_{qi}_{qb}")
                nc.vector.tensor_mul(u, s1, lam_sb)
                r2q = smallp.tile([P, 1], F32, tag="r2q", name=f"r2q_{qi}_{qb}")
                nc.vector.tensor_mul(r2q, u, r2)
                pend.append((qb, ets, r2q))

            def emit_combine():
                qb, ets, r2q = pend.pop(0)
                # p = e1 - e2 * r2q   (into ets[0]); t on ACT, sub on DVE
                for kn in range(SN):
                    ks = ts(kn, 512)
                    t2 = tmpp.tile([P, 512], F32, tag="t2", name=f"t2_{qi}_{qb}_{kn}")
                    nc.vector.tensor_scalar_mul(t2, ets[1][:, ks], r2q)
                    nc.vector.tensor_sub(ets[0][:, ks], ets[0][:, ks], t2)
                # transpose p into ptile[:, :, qb-block]; batch 4 transposes
                # into one psum bank, evict with one strided copy
                for kc4 in range(NKC // 4):
                    tp = wps.tile([P, 4, P], F32R, tag="ps", name=f"tp_{qi}_{qb}_{kc4}")
                    for j in range(4):
                        kc = kc4 * 4 + j
                        nc.tensor.matmul(tp[:, j], ets[0][:, ts(kc, P)], ident,
                                         is_transpose=True)
                    nc.vector.tensor_copy(ptile[:, ts(kc4, 4), ts(qb, P)], tp)

            # software pipeline: scores(qb+1) sits ahead of combine(qb) in the
            # PE queue so the PE never stalls on the ACT/DVE combine tail
            for qb in range(QB):
                emit_scores(qb)
                if qb > 0:
                    emit_combine()
            emit_combine()

            if phases == "scores":
                dbg2 = ofp.tile([P, DM], F32, tag="of", name=f"dbg2_{qi}")
                nc.vector.tensor_copy(dbg2, ptile[:, 0, :DM].bitcast(F32))
                nc.sync.dma_start(out[ds(qi * QC, P), :], dbg2)
                continue

            # attn @ V (transposed out) and out-proj, accumulated over v-chunks;
            # same trick: oT(vc+1) accumulation is queued before outF(vc)
            fts = [fps.tile([P, DM], F32, tag="f", name=f"f_{qi}_{q}") for q in range(QB)]
            ot_pend = []

            def emit_ot(vc):
                vp = vpp.tile([P, NKC, P], F32R, tag="vp", name=f"vp_{qi}_{vc}")
                nc.sync.dma_start(vp, vd[vc])
                ot_ps = wps.tile([P, 512], F32, tag="ps", name=f"otps_{qi}_{vc}")
                for kc in range(NKC):
                    nc.tensor.matmul(
                        ot_ps, vp[:, kc, :], ptile[:, kc, :],
                        start=(kc == 0), stop=(kc == NKC - 1))
                ot_sb = otp.tile([P, 512], F32R, tag="ot", name=f"ot_{qi}_{vc}")
                nc.vector.tensor_copy(ot_sb, ot_ps)
                ot_pend.append((vc, ot_sb))

            def emit_outf():
                vc, ot_sb = ot_pend.pop(0)
                for qs in range(QB):
                    nc.tensor.matmul(
                        fts[qs], ot_sb[:, ts(qs, P)], woT[:, vc, :],
                        start=(vc == 0), stop=(vc == NVC - 1))

            for vc in range(NVC):
                emit_ot(vc)
                if vc > 0:
                    emit_outf()
            emit_outf()
            for qs in range(QB):
                ofsb = ofp.tile([P, DM], F32, tag="of", name=f"of_{qi}_{qs}")
                nc.scalar.activation(ofsb, fts[qs], IDENT, scale=r1s[qs])
                nc.sync.dma_start(out[ds(qi * QC + qs * P, P), :], ofsb)


def build_module(n_iters=1, phases="full"):
    nc = bacc.Bacc("TRN2", target_bir_lowering=False, debug=False)
    xT = nc.dram_tensor("xT", (D, S), F32R, kind="ExternalInput").ap()
    wqkv = nc.dram_tensor("wqkv", (D, 2 * D), F32R, kind="ExternalInput").ap()
    wv = nc.dram_tensor("wv", (D, VS), F32R, kind="ExternalInput").ap()
    wo = nc.dram_tensor("wo", (VS, DM), F32R, kind="ExternalInput").ap()
    lam = nc.dram_tensor("lam", (P, 1), F32, kind="ExternalInput").ap()
    bq = nc.dram_tensor("bq", (P, MQ), F32, kind="ExternalInput").ap()
    out = nc.dram_tensor("out", (S, DM), F32, kind="ExternalOutput").ap()
    with tile.TileContext(nc) as tc:
        for _ in range(n_iters):
            kernel_body(tc, xT, wqkv, wv, wo, lam, bq, out, phases)
    nc.compile()
    return nc


_NC = None


def _get_module():
    global _NC
    if _NC is None:
        _NC = build_module()
    return _NC


def host_prep(**inputs):
    """Host-side input prep: returns (in_maps, lam, host_bias)."""
    x = np.asarray(inputs["x"], np.float32)
    Wqkv = np.asarray(inputs["Wqkv"], np.float32)
    bqkv = np.asarray(inputs["bqkv"], np.float32)
    Wv = np.asarray(inputs["Wv"], np.float32)
    bv = np.asarray(inputs["bv"], np.float32)
    Wo = np.asarray(inputs["Wo"], np.float32)
    bo = np.asarray(inputs["bo"], np.float32)
    lq1 = np.asarray(inputs["lq1"], np.float32)
    lk1 = np.asarray(inputs["lk1"], np.float32)
    lq2 = np.asarray(inputs["lq2"], np.float32)
    lk2 = np.asarray(inputs["lk2"], np.float32)

    lam = float(
        np.exp(np.sum(lq1 * lk1, dtype=np.float32))
        - np.exp(np.sum(lq2 * lk2, dtype=np.float32))
        + (LAMBDA_INIT - 0.6 * math.exp(-0.3 * LAYER_INDEX))
    )
    bq_host = np.ascontiguousarray(bqkv.reshape(MQ, P).T)
    lam_host = np.full((P, 1), lam, np.float32)

    in_maps = []
    for c in range(8):
        b, v = divmod(c, NV)
        in_maps.append({
            "xT": np.ascontiguousarray(x[b].T),
            "wqkv": np.ascontiguousarray(Wqkv),
            "wv": np.ascontiguousarray(Wv[:, v * VS : (v + 1) * VS]),
            "wo": np.ascontiguousarray(Wo[v * VS : (v + 1) * VS, :]),
            "lam": lam_host,
            "bq": bq_host,
        })
    # sum_k diff_attn[q, :] == 1 - lam exactly, so bv and bo fold into a
    # constant per-output-column correction.
    host_bias = ((1.0 - lam) * bv) @ Wo + bo
    return in_maps, lam, host_bias.astype(np.float32)


def kernel(**inputs):
    in_maps, _lam, host_bias = host_prep(**inputs)
    nc = _get_module()
    res = bass_utils.run_bass_kernel_spmd(nc, in_maps, core_ids=list(range(8)))
    out = np.zeros((B, S, DM), np.float32)
    for c in range(8):
        b, _v = divmod(c, NV)
        out[b] += res.results[c]["out"]
    out += host_bias
    return out



# revision 2
# speedup vs baseline: 20.9825x; 20.9825x over previous
"""Differential attention kernel for Trainium2 (8 NeuronCores).

Key identity: everything after the differential combine is linear, so
    out = diff_attn @ (x @ Wv) @ Wo + bias  ==  diff_attn @ x @ (Wv @ Wo) + bias
W2 = Wv @ Wo is a [512, 512] weight-only product folded on the host, which
removes the 8192-wide V projection / attn@V / out_proj entirely.

Sharding: 2 batches x 4 query-quarters (512 queries per core). Each core
computes full K (and V2 = x @ W2) for its batch plus Q for its own quarter,
its 512x2048 slice of both attention maps, and the 512x512 output slice.
The three big per-core GEMM groups (K/V2/scores/attn@V2) have no cross-core
duplication except K and V2 (shared per batch).

All matmuls run as float32r (fp32 storage, full-rate PE mode). Softmax is
computed without max-subtraction (|logits| < ~12, safe in fp32). The 1/sum(e1)
normalizer is folded into the final PSUM eviction, so the combine is a single
fused DVE op computing -p = e2*(lam*s1/s2) - e1; the sign is absorbed by
evicting with scale = -1/s1. bqkv is applied via the ScalarE bias port;
bv/bo are folded exactly on the host using sum_k(diff_attn[q,:]) == 1 - lam.
"""

import math

import numpy as np

import concourse.bass as bass
from concourse import bacc
import concourse.mybir as mybir
import concourse.tile as tile
from concourse import bass_utils
from concourse.bass import ts, ds
from concourse.masks import make_identity

# Problem shapes (hardcoded per harness contract).
B = 2
S = 2048
D = 512
DM = 512             # output dim
P = 128
QC = 512             # queries per core
NQ = S // QC         # 4 q-shards per batch
SCALE = 1.0 / math.sqrt(64.0)
LAMBDA_INIT = 0.8
LAYER_INDEX = 0

F32 = mybir.dt.float32
F32R = mybir.dt.float32r
EXP = mybir.ActivationFunctionType.Exp
IDENT = mybir.ActivationFunctionType.Identity
AXX = mybir.AxisListType.X
MUL = mybir.AluOpType.mult
SUB = mybir.AluOpType.subtract

KD = D // P          # 4 contraction chunks of the input dim
MQ = (2 * D) // P    # 8 bias columns (Q: 0..3, K: 4..7)
SN = S // 512        # 4 key chunks of 512
NKC = S // P         # 16 key chunks of 128
QB = QC // P         # 4 q-blocks per core


def kernel_body(tc, xT, xTq, wqkv, w2, lam, bq, out):
    nc = tc.nc
    # tolerate f32-typed dram tensors (e.g. run_kernel's sim harness)
    if xT.dtype != F32R:
        xT = xT.bitcast(F32R)
    if xTq.dtype != F32R:
        xTq = xTq.bitcast(F32R)
    if wqkv.dtype != F32R:
        wqkv = wqkv.bitcast(F32R)
    if w2.dtype != F32R:
        w2 = w2.bitcast(F32R)

    with tc.tile_pool(name="persist", bufs=1) as persist:
        kT = persist.tile([P, 4, S], F32R)      # K1,K2 transposed: [feat, k]
        qT = persist.tile([P, 4, QC], F32R)     # Q1,Q2 own slice:  [feat, q]
        v2 = persist.tile([P, NKC, DM], F32R)   # V2 rows: [s-in-block, kc, dm]
        lam_sb = persist.tile([P, 1], F32)
        bq_sb = persist.tile([P, MQ], F32)
        ident_f32 = persist.tile([P, P], F32)
        ident = persist.tile([P, P], F32R)

        nc.sync.dma_start(lam_sb, lam)
        nc.sync.dma_start(bq_sb, bq)
        make_identity(nc, ident_f32)
        nc.vector.tensor_copy(ident, ident_f32)

        # ---------------- setup: Q, K, V2 projections ----------------
        with (
            tc.tile_pool(name="setup", bufs=1) as setup,
            tc.tile_pool(name="spsum", bufs=6, space="PSUM") as spsum,
        ):
            xTs = setup.tile([P, KD, S], F32R)
            xTqs = setup.tile([P, KD, QC], F32R)
            wq_sb = setup.tile([P, KD, 2 * D], F32R)
            w2_sb = setup.tile([P, KD, DM], F32R)

            # loads in first-consumption order: Q proj, then K proj, then V2
            for dc in range(KD):
                nc.sync.dma_start(xTqs[:, dc], xTq[ds(dc * P, P), :])
                nc.sync.dma_start(wq_sb[:, dc, :D], wqkv[ds(dc * P, P), :D])
            for dc in range(KD):
                nc.sync.dma_start(wq_sb[:, dc, D:], wqkv[ds(dc * P, P), D:])
                nc.sync.dma_start(xTs[:, dc, ts(0, 512)], xT[ds(dc * P, P), ts(0, 512)])
            for sn in range(1, SN):
                for dc in range(KD):
                    nc.sync.dma_start(xTs[:, dc, ts(sn, 512)],
                                      xT[ds(dc * P, P), ts(sn, 512)])
            for dc in range(KD):
                nc.sync.dma_start(w2_sb[:, dc], w2[ds(dc * P, P), :])

            # Q proj (own slice): qT[m*128+p, q] = sum_d Wqkv[d, m*128+p] xTq[d, q]
            for m in range(4):
                pt = spsum.tile([P, QC], F32, tag="ps")
                for dc in range(KD):
                    nc.tensor.matmul(pt, wq_sb[:, dc, ts(m, P)], xTqs[:, dc],
                                     start=(dc == 0), stop=(dc == KD - 1))
                nc.scalar.activation(qT[:, m], pt, IDENT, bias=bq_sb[:, m : m + 1])
            # K proj (full batch)
            for sn in range(SN):
                for m in range(4):
                    pt = spsum.tile([P, 512], F32, tag="ps")
                    for dc in range(KD):
                        nc.tensor.matmul(pt, wq_sb[:, dc, ts(4 + m, P)],
                                         xTs[:, dc, ts(sn, 512)],
                                         start=(dc == 0), stop=(dc == KD - 1))
                    nc.scalar.activation(kT[:, m, ts(sn, 512)], pt, IDENT,
                                         bias=bq_sb[:, 4 + m : 5 + m])
            # V2 = x @ W2 (full batch), rows grouped by 128-key block;
            # evictions on DVE so ACT is free for the first exp tiles
            for kc in range(NKC):
                pt = spsum.tile([P, DM], F32, tag="ps")
                for dc in range(KD):
                    nc.tensor.matmul(pt, xTs[:, dc, ts(kc, P)], w2_sb[:, dc],
                                     start=(dc == 0), stop=(dc == KD - 1))
                nc.vector.tensor_copy(v2[:, kc], pt)

        # ---------------- main: attention for own 512 queries ----------------
        with (
            tc.tile_pool(name="e1p", bufs=2) as e1p,
            tc.tile_pool(name="e2p", bufs=2) as e2p,
            tc.tile_pool(name="smallp", bufs=3) as smallp,
            tc.tile_pool(name="r1p", bufs=QB) as r1p,
            tc.tile_pool(name="ptp", bufs=1) as ptp,
            tc.tile_pool(name="ofp", bufs=2) as ofp,
            tc.tile_pool(name="wps", bufs=4, space="PSUM") as wps,
            tc.tile_pool(name="fps", bufs=2, space="PSUM") as fps,
        ):
            ptile = ptp.tile([P, NKC, QC], F32R)
            r1s = [None] * QB
            pend = []

            def emit_scores(qb):
                ets = []
                sums = []
                for mi in range(2):
                    pool = e1p if mi == 0 else e2p
                    et = pool.tile([P, S], F32R, tag=f"e{mi}", name=f"e{mi}_{qb}")
                    st = smallp.tile([P, SN], F32, tag=f"sum{mi}",
                                     name=f"sum{mi}_{qb}")
                    for kn in range(SN):
                        pt = wps.tile([P, 512], F32, tag="ps",
                                      name=f"ps_{qb}_{mi}_{kn}")
                        for dc in range(2):
                            nc.tensor.matmul(
                                pt,
                                qT[:, 2 * mi + dc, ts(qb, P)],
                                kT[:, 2 * mi + dc, ts(kn, 512)],
                                start=(dc == 0), stop=(dc == 1))
                        nc.scalar.activation(
                            et[:, ts(kn, 512)], pt, EXP, scale=SCALE,
                            accum_out=st[:, kn : kn + 1])
                    ets.append(et)
                    sums.append(st)
                # normalizers: final evict uses -1/s1; combine uses lam*s1/s2
                s1 = smallp.tile([P, 1], F32, tag="s1", name=f"s1_{qb}")
                nc.vector.reduce_sum(s1, sums[0], axis=AXX)
                s1n = smallp.tile([P, 1], F32, tag="s1n", name=f"s1n_{qb}")
                nc.vector.tensor_scalar_mul(s1n, s1, -1.0)
                r1n = r1p.tile([P, 1], F32, tag="r1", name=f"r1_{qb}")
                nc.vector.reciprocal(r1n, s1n)
                r1s[qb] = r1n
                s2 = smallp.tile([P, 1], F32, tag="s2", name=f"s2_{qb}")
                nc.vector.reduce_sum(s2, sums[1], axis=AXX)
                r2 = smallp.tile([P, 1], F32, tag="r2", name=f"r2_{qb}")
                nc.vector.reciprocal(r2, s2)
                u = smallp.tile([P, 1], F32, tag="u", name=f"u_{qb}")
                nc.vector.tensor_mul(u, s1, lam_sb)
                r2q = smallp.tile([P, 1], F32, tag="r2q", name=f"r2q_{qb}")
                nc.vector.tensor_mul(r2q, u, r2)
                pend.append((qb, ets, r2q))

            def emit_combine_attn():
                qb, ets, r2q = pend.pop(0)
                # -p = e2*r2q - e1, one fused DVE pass, in place into e1
                nc.vector.scalar_tensor_tensor(
                    ets[0][:, :], ets[1][:, :], r2q, ets[0][:, :], MUL, SUB)
                # transpose -p into ptile[:, :, qb-block]; batch 4 transposes
                # per psum bank, evict with one strided copy
                for kc4 in range(NKC // 4):
                    tp = wps.tile([P, 4, P], F32R, tag="ps",
                                  name=f"tp_{qb}_{kc4}")
                    for j in range(4):
                        kc = kc4 * 4 + j
                        nc.tensor.matmul(tp[:, j], ets[0][:, ts(kc, P)], ident,
                                         is_transpose=True)
                    nc.vector.tensor_copy(ptile[:, ts(kc4, 4), ts(qb, P)], tp)
                # (-p)^T @ V2, evicted with scale -1/s1
                ft = fps.tile([P, DM], F32, tag="f", name=f"f_{qb}")
                for kc in range(NKC):
                    nc.tensor.matmul(ft, ptile[:, kc, ts(qb, P)], v2[:, kc, :],
                                     start=(kc == 0), stop=(kc == NKC - 1))
                of = ofp.tile([P, DM], F32, tag="of", name=f"of_{qb}")
                nc.scalar.activation(of, ft, IDENT, scale=r1s[qb])
                nc.sync.dma_start(out[ds(qb * P, P), :], of)

            # software pipeline: scores(qb+1) sits ahead of combine(qb) in the
            # PE queue so the PE never stalls on the ACT/DVE combine tail
            for qb in range(QB):
                emit_scores(qb)
                if qb > 0:
                    emit_combine_attn()
            emit_combine_attn()


def build_module(n_iters=1):
    nc = bacc.Bacc("TRN2", target_bir_lowering=False, debug=False)
    xT = nc.dram_tensor("xT", (D, S), F32R, kind="ExternalInput").ap()
    xTq = nc.dram_tensor("xTq", (D, QC), F32R, kind="ExternalInput").ap()
    wqkv = nc.dram_tensor("wqkv", (D, 2 * D), F32R, kind="ExternalInput").ap()
    w2 = nc.dram_tensor("w2", (D, DM), F32R, kind="ExternalInput").ap()
    lam = nc.dram_tensor("lam", (P, 1), F32, kind="ExternalInput").ap()
    bq = nc.dram_tensor("bq", (P, MQ), F32, kind="ExternalInput").ap()
    out = nc.dram_tensor("out", (QC, DM), F32, kind="ExternalOutput").ap()
    with tile.TileContext(nc) as tc:
        for _ in range(n_iters):
            kernel_body(tc, xT, xTq, wqkv, w2, lam, bq, out)
    nc.compile()
    return nc


_NC = None


def _get_module():
    global _NC
    if _NC is None:
        _NC = build_module()
    return _NC


def host_prep(**inputs):
    """Host-side input prep: returns (in_maps, lam, host_bias)."""
    x = np.asarray(inputs["x"], np.float32)
    Wqkv = np.asarray(inputs["Wqkv"], np.float32)
    bqkv = np.asarray(inputs["bqkv"], np.float32)
    Wv = np.asarray(inputs["Wv"], np.float32)
    bv = np.asarray(inputs["bv"], np.float32)
    Wo = np.asarray(inputs["Wo"], np.float32)
    bo = np.asarray(inputs["bo"], np.float32)
    lq1 = np.asarray(inputs["lq1"], np.float32)
    lk1 = np.asarray(inputs["lk1"], np.float32)
    lq2 = np.asarray(inputs["lq2"], np.float32)
    lk2 = np.asarray(inputs["lk2"], np.float32)

    lam = float(
        np.exp(np.sum(lq1 * lk1, dtype=np.float32))
        - np.exp(np.sum(lq2 * lk2, dtype=np.float32))
        + (LAMBDA_INIT - 0.6 * math.exp(-0.3 * LAYER_INDEX))
    )
    bq_host = np.ascontiguousarray(bqkv.reshape(MQ, P).T)
    lam_host = np.full((P, 1), lam, np.float32)
    # weight-only fold: out_proj absorbs the V projection
    W2 = np.ascontiguousarray(Wv @ Wo)

    in_maps = []
    for c in range(8):
        b, qs = divmod(c, NQ)
        xb = x[b]
        in_maps.append({
            "xT": np.ascontiguousarray(xb.T),
            "xTq": np.ascontiguousarray(xb[qs * QC : (qs + 1) * QC].T),
            "wqkv": np.ascontiguousarray(Wqkv),
            "w2": W2,
            "lam": lam_host,
            "bq": bq_host,
        })
    # sum_k diff_attn[q, :] == 1 - lam exactly, so bv and bo fold into a
    # constant per-output-column correction.
    host_bias = ((1.0 - lam) * bv) @ Wo + bo
    return in_maps, lam, host_bias.astype(np.float32)


def kernel(**inputs):
    in_maps, _lam, host_bias = host_prep(**inputs)
    nc = _get_module()
    res = bass_utils.run_bass_kernel_spmd(nc, in_maps, core_ids=list(range(8)))
    out = np.empty((B, S, DM), np.float32)
    for c in range(8):
        b, qs = divmod(c, NQ)
        out[b, qs * QC : (qs + 1) * QC, :] = res.results[c]["out"]
    out += host_bias
    return out
